# revision 31
# baseline (speedup 1.0000x reference)
"""Multi-head attention (B=2, S=4096, D=1024, H=16) on 8 NeuronCores.

Sharding: core c = (batch b = c // 4, head-group g = c % 4).  Each head-group
owns 4 heads = 256 projection features.  All device compute in bf16
(fp32 PSUM accumulation); host pre-transposes and casts inputs so the
kernel does zero on-chip transposition of activations or weights:
  - host supplies qT/kT/vT [D, S] bf16, wqT/wkT/wvT [D, E] bf16,
    w0T [E, D] bf16 per core
  - projections contract over d in 8 chunks of 128 (fp32 PSUM)
  - attention per head with scores transposed ([k, q]); softmax
    normalization deferred via a ones column in the PV stationary operand
    (row 64 of the PV output = exp row-sums); exp computes
    exp(score/8 - 2) on ACT -- the -2 bias cancels in the softmax ratio
  - per-head normalization (transpose -> scale by 1/sum -> transpose back)
  - output projection -> partial [S, D] bf16; host sums the 4 partials
    per batch in fp32.

Performance notes (measured on hardware):
  - The PE HAM clock gate only reaches K=8/8 (2.4 GHz) when matmuls use
    full [128,128] stationary tiles AND the PE has no idle gaps.  QK
    therefore uses a zero-padded 2-head stationary (kpTz) against the
    full 2-head qpT moving operand, and PV pads the vps stationary to
    128 columns (output rows 65..127 are ignored).  This alone is ~1.5x.
  - Phase A is software-pipelined (QK_k, exp_k issued before PV_{k-1})
    with st triple-buffered so neither PE nor ACT ever waits; the phase
    runs at the ACT exp roofline (~1.15us per 128x1024 stripe).
  - A ~16us dummy-matmul warm-up burst keeps the PE busy while the first
    DMAs land so phase T starts warm.
  - fp8 (DoubleRow) and Schraudolph-exp offloads were evaluated and
    rejected: both push max-rel-err to ~0.02 (the correctness gate).
"""

import numpy as np
from contextlib import ExitStack

import concourse.bass as bass
import concourse.bacc as bacc
import concourse.tile as tile
from concourse import mybir, bass_utils
from concourse.masks import make_identity
import ml_dtypes

B, S, D, H = 2, 4096, 1024, 16
DK = D // H          # 64
NCORES = 8
GROUPS = 4           # head-groups (tensor parallel)
HG = H // GROUPS     # 4 heads per group
E = HG * DK          # 256 features per group

F32 = mybir.dt.float32
BF16 = mybir.dt.bfloat16
NPBF16 = ml_dtypes.bfloat16

P = 128              # partitions
DC = D // P          # 8 d-chunks
SC = S // P          # 32 s-chunks of 128
SW = 1024            # projection staging window along s
NW = S // SW         # 4
QB = 1024            # q-block in attention
NQB = S // QB        # 4
NST = SC             # 32 k-stripes of 128
EXP_BIAS = -2.0      # exp(s/8 - 2): keeps exp outputs small; cancels in ratio


def kernel_body(tc, qT, kT, vT, wqT, wkT, wvT, w0T, out):
    nc = tc.nc
    ctx = ExitStack()
    with ctx:
        ident_pool = ctx.enter_context(tc.tile_pool(name="ident", bufs=1))
        identity = ident_pool.tile([P, P], F32)
        make_identity(nc, identity)
        ebias = ident_pool.tile([P, 1], F32, tag="ebias", name="ebias")
        nc.vector.memset(ebias, EXP_BIAS)
        ident_bf = ident_pool.tile([P, P], BF16, tag="ident_bf",
                                   name="ident_bf")
        make_identity(nc, ident_bf)

        # PE warm-up: ~5us of full-tile matmuls on dummy data so the HAM
        # clock gate reaches K=8/8 while the first input DMAs land.
        with tc.tile_pool(name="warm", bufs=1) as warm_pool, \
             tc.tile_pool(name="warm_ps", bufs=1, space="PSUM") as warm_ps:
            wdat = warm_pool.tile([P, 512], BF16, tag="wdat", name="wdat")
            nc.vector.memset(wdat, 1.0)
            wps = warm_ps.tile([P, 512], F32, tag="wps", name="wps")
            NWARM = 84   # ~18us: keeps PE busy until the first DMAs land
            for i in range(NWARM):
                nc.tensor.matmul(wps, wdat[:, 0:P], wdat,
                                 start=(i == 0), stop=(i == NWARM - 1))

        # persistent across A..W
        w0s_pool = ctx.enter_context(tc.tile_pool(name="w0s", bufs=1))
        w0s = w0s_pool.tile([P, 2, D], BF16, tag="w0s", name="w0s")
        for ec in range(2):
            nc.sync.dma_start(out=w0s[:, ec, :], in_=w0T[ec * P:(ec + 1) * P, :])

        # persistent through phase A
        proj_ctx = ExitStack()
        proj_pool = proj_ctx.enter_context(tc.tile_pool(name="proj", bufs=1))
        qpT = [proj_pool.tile([P, S], BF16, tag=f"qpT{i}", name=f"qpT{i}")
               for i in range(2)]
        # kpTz[et][:, hh, :]: zero-padded per-head stationary for QK -- rows
        # hh*64..hh*64+63 hold kp for head 2*et+hh, the other 64 rows are
        # zero, so QK can use a full [128,128] stationary tile (keeps the PE
        # HAM activity monitor warm) against the full 2-head qpT moving
        # operand.
        kpTz = [proj_pool.tile([P, 2, S], BF16, tag=f"kpTz{i}", name=f"kpTz{i}")
                for i in range(2)]
        # vps[:, sc, h, :]: cols 0-63 vp, col 64 ones (softmax row-sums),
        # cols 65-127 duplicate vp data (padding so the PV stationary is a
        # full [128,128] tile; PV output rows 65-127 are ignored).
        vps = proj_pool.tile([P, SC, HG, P], BF16, tag="vps", name="vps")

        # ================= phase T: load + projections =================
        with tc.tile_pool(name="t_w", bufs=1) as wpool, \
             tc.tile_pool(name="t_x", bufs=2) as xpool, \
             tc.tile_pool(name="t_ps", bufs=4, space="PSUM") as pspool:
            wqs = wpool.tile([P, DC, E], BF16, tag="wqs", name="wqs")
            wks = wpool.tile([P, DC, E], BF16, tag="wks", name="wks")
            wvs = wpool.tile([P, DC, E], BF16, tag="wvs", name="wvs")
            for et in range(2):
                nc.vector.memset(kpTz[et], 0.0)
            for wsrc, wdst in ((wqT, wqs), (wkT, wks), (wvT, wvs)):
                for dc in range(DC):
                    nc.sync.dma_start(out=wdst[:, dc, :],
                                      in_=wsrc[dc * P:(dc + 1) * P, :])

            for w in range(NW):
                s0 = w * SW
                for kind, src in ((0, qT), (1, kT), (2, vT)):
                    xst = xpool.tile([P, DC, SW], BF16, tag="xst", name="xst")
                    for dc in range(DC):
                        nc.sync.dma_start(
                            out=xst[:, dc, :],
                            in_=src[dc * P:(dc + 1) * P, s0:s0 + SW])
                    if kind < 2:
                        wT = wqs if kind == 0 else wks
                        for et in range(2):
                            for sb in range(SW // 512):
                                acc = pspool.tile([P, 512], F32, tag="acc",
                                                  name="acc")
                                for dc in range(DC):
                                    nc.tensor.matmul(
                                        acc,
                                        wT[:, dc, et * P:(et + 1) * P],
                                        xst[:, dc, sb * 512:(sb + 1) * 512],
                                        start=(dc == 0), stop=(dc == DC - 1))
                                sl = slice(s0 + sb * 512, s0 + (sb + 1) * 512)
                                if kind == 0:
                                    nc.vector.tensor_copy(
                                        out=qpT[et][:, sl], in_=acc)
                                else:
                                    nc.vector.tensor_copy(
                                        out=kpTz[et][0:DK, 0, sl],
                                        in_=acc[0:DK, :])
                                    nc.vector.tensor_copy(
                                        out=kpTz[et][DK:P, 1, sl],
                                        in_=acc[DK:P, :])
                    else:
                        for sc4 in range(SW // P):
                            scg = w * (SW // P) + sc4
                            accv = pspool.tile([P, E], F32, tag="accv",
                                               name="accv")
                            for dc in range(DC):
                                nc.tensor.matmul(
                                    accv,
                                    xst[:, dc, sc4 * P:(sc4 + 1) * P],
                                    wvs[:, dc, :],
                                    start=(dc == 0), stop=(dc == DC - 1))
                            nc.vector.tensor_copy(
                                out=vps[:, scg, :, 0:DK],
                                in_=accv.rearrange("p (h c) -> p h c", c=DK))
                            # duplicate vp into the pad columns (65..127)
                            nc.vector.tensor_copy(
                                out=vps[:, scg, :, DK + 1:P],
                                in_=accv.rearrange(
                                    "p (h c) -> p h c", c=DK)[:, :, 0:P - DK - 1])
            # ones column for the PV sums row
            ones_sc = wpool.tile([P, SC], F32, tag="ones_sc", name="ones_sc")
            nc.vector.memset(ones_sc, 1.0)
            for h in range(HG):
                nc.vector.tensor_copy(
                    out=vps[:, :, h, DK:DK + 1],
                    in_=ones_sc.rearrange("p (s o) -> p s o", o=1))

        # ================= phase A: attention =================
        x65_pool = ctx.enter_context(
            tc.tile_pool(name="x65", bufs=1, side="right"))
        x65 = [x65_pool.tile([P, S], BF16, tag=f"x65_{h}", name=f"x65_{h}")
               for h in range(HG)]
        with tc.tile_pool(name="a_att", bufs=4) as att_pool, \
             tc.tile_pool(name="a_st", bufs=3, space="PSUM") as ppool_st, \
             tc.tile_pool(name="a_x", bufs=1, space="PSUM") as ppool_x:
            # Software-pipelined: issue QK_k and exp_k, then PV_{k-1}, so the
            # PE never sits waiting on the QK->exp->PV semaphore chain (a PE
            # idle gap each stripe keeps the HAM clock gate at K=4/8 =
            # 1.2 GHz; saturating PE unthrottles it to 2.4 GHz).
            for h in range(HG):
                et, hh = h // 2, h % 2
                for qb in range(NQB):
                    q0 = qb * QB
                    xacc = ppool_x.tile([P, QB], F32, tag="xacc", name="xacc")
                    attq = []  # pending attst tiles for PV
                    for kk in range(NST):
                        st = ppool_st.tile([P, QB], F32, tag="st", name="st")
                        lhs_k = kpTz[et][:, hh, kk * P:(kk + 1) * P]
                        for j in range(QB // 512):
                            nc.tensor.matmul(
                                st[:, j * 512:(j + 1) * 512],
                                lhs_k,
                                qpT[et][:, q0 + j * 512:q0 + (j + 1) * 512],
                                start=True, stop=True)
                        attst = att_pool.tile([P, QB], BF16, tag="att",
                                              name="att")
                        nc.scalar.activation(
                            attst, st, mybir.ActivationFunctionType.Exp,
                            bias=ebias, scale=0.125)
                        attq.append((kk, attst))
                        if kk > 0:
                            pkk, patt = attq.pop(0)
                            lv = vps[:, pkk, h, :]
                            for j in range(QB // 512):
                                nc.tensor.matmul(
                                    xacc[:, j * 512:(j + 1) * 512],
                                    lv,
                                    patt[:, j * 512:(j + 1) * 512],
                                    start=(pkk == 0), stop=False)
                    pkk, patt = attq.pop(0)
                    lv = vps[:, pkk, h, :]
                    for j in range(QB // 512):
                        nc.tensor.matmul(
                            xacc[:, j * 512:(j + 1) * 512],
                            lv,
                            patt[:, j * 512:(j + 1) * 512],
                            start=False, stop=True)
                    nc.vector.tensor_copy(out=x65[h][:, q0:q0 + QB],
                                          in_=xacc)
        proj_ctx.close()   # release qpT/kpT/vps

        # ===== phase NW: normalize + project + store =====
        # Batched per group of 4 q-chunks with packed PSUM tiles so the
        # in-order PE queue never stalls on the DVE/ACT chain: all forward
        # transposes of a group issue together, then the element ops, then
        # the transpose-backs of the previous group, then W matmuls.
        GQ = 4  # q-chunks per group
        xw_pool = ctx.enter_context(
            tc.tile_pool(name="xw", bufs=1, side="right"))
        xw = [xw_pool.tile([P, S], BF16, tag=f"xw{i}", name=f"xw{i}")
              for i in range(2)]
        xs2all = xw_pool.tile([P, SC, 2, 2 * DK], BF16, tag="xs2all",
                              name="xs2all")
        with tc.tile_pool(name="n_sb", bufs=8) as nsb_pool, \
             tc.tile_pool(name="n_ps", bufs=4, space="PSUM") as ppool_n:
            # stage 1: transpose all 4 heads of a q-chunk into one packed
            # PSUM tile, one strided reciprocal for all 4 sums, 4 scaled
            # writes into xs2 (DVE for et0, ACT for et1)
            xs2s = {}
            for qc in range(SC):
                tps = ppool_n.tile([P, HG, P], BF16, tag="ntp", name="ntp")
                for h in range(HG):
                    nc.tensor.transpose(
                        tps[:, h, :], x65[h][:, qc * P:(qc + 1) * P], ident_bf)
                rcp4 = nsb_pool.tile([P, HG], F32, tag="rcp4", name="rcp4")
                nc.vector.reciprocal(rcp4, tps[:, :, DK:DK + 1])
                for et in range(2):
                    xs2 = xs2all[:, qc, et, :]
                    xs2s[(qc, et)] = xs2
                    for hp2 in range(2):
                        h = 2 * et + hp2
                        if et == 0:
                            nc.vector.tensor_scalar_mul(
                                xs2[:, hp2 * DK:(hp2 + 1) * DK],
                                tps[:, h, 0:DK], rcp4[:, h:h + 1])
                        else:
                            nc.scalar.mul(
                                xs2[:, hp2 * DK:(hp2 + 1) * DK],
                                tps[:, h, 0:DK], rcp4[:, h:h + 1])
        with tc.tile_pool(name="w_sb", bufs=3) as osb_pool, \
             tc.tile_pool(name="nb_ps", bufs=3, space="PSUM") as ppool_nb, \
             tc.tile_pool(name="w_ps", bufs=2, space="PSUM") as ppool_w:
            # stage 2: transpose back (both et packed per PSUM tile)
            for qc in range(SC):
                tbs = ppool_nb.tile([P, 2, P], BF16, tag="ntb", name="ntb")
                for et in range(2):
                    nc.tensor.transpose(tbs[:, et, :], xs2s[(qc, et)],
                                        ident_bf)
                nc.vector.tensor_copy(
                    out=xw[0][:, qc * P:(qc + 1) * P], in_=tbs[:, 0, :])
                nc.scalar.copy(
                    out=xw[1][:, qc * P:(qc + 1) * P], in_=tbs[:, 1, :])
            # stage 3: output projection + DMA out
            for qc in range(SC):
                oacc = ppool_w.tile([P, D], F32, tag="oacc", name="oacc")
                for ec in range(2):
                    for j in range(2):
                        nc.tensor.matmul(
                            oacc[:, j * 512:(j + 1) * 512],
                            xw[ec][:, qc * P:(qc + 1) * P],
                            w0s[:, ec, j * 512:(j + 1) * 512],
                            start=(ec == 0), stop=(ec == 1))
                osb = osb_pool.tile([P, D], BF16, tag="osb", name="osb")
                nc.vector.tensor_copy(out=osb, in_=oacc)
                nc.sync.dma_start(out=out[qc * P:(qc + 1) * P, :], in_=osb)


def build_program():
    nc = bacc.Bacc("TRN2", target_bir_lowering=False, debug=False,
                   num_devices=NCORES)
    qT = nc.dram_tensor("qT", (D, S), BF16, kind="ExternalInput").ap()
    kT = nc.dram_tensor("kT", (D, S), BF16, kind="ExternalInput").ap()
    vT = nc.dram_tensor("vT", (D, S), BF16, kind="ExternalInput").ap()
    wqT = nc.dram_tensor("wqT", (D, E), BF16, kind="ExternalInput").ap()
    wkT = nc.dram_tensor("wkT", (D, E), BF16, kind="ExternalInput").ap()
    wvT = nc.dram_tensor("wvT", (D, E), BF16, kind="ExternalInput").ap()
    w0T = nc.dram_tensor("w0T", (E, D), BF16, kind="ExternalInput").ap()
    out = nc.dram_tensor("out", (S, D), BF16, kind="ExternalOutput").ap()
    with tile.TileContext(nc) as tc:
        kernel_body(tc, qT, kT, vT, wqT, wkT, wvT, w0T, out)
    nc.compile()
    return nc


_NC_CACHE = None


def _get_program():
    global _NC_CACHE
    if _NC_CACHE is None:
        _NC_CACHE = build_program()
    return _NC_CACHE


def make_in_maps(q, k, v, wq, wk, wv, w0):
    arrs = [np.asarray(a, dtype=np.float32)
            for a in (q, k, v, wq, wk, wv, w0)]
    q, k, v, wq, wk, wv, w0 = arrs
    qTb = [np.ascontiguousarray(q[b].astype(NPBF16).T) for b in range(B)]
    kTb = [np.ascontiguousarray(k[b].astype(NPBF16).T) for b in range(B)]
    vTb = [np.ascontiguousarray(v[b].astype(NPBF16).T) for b in range(B)]
    in_maps = []
    for c in range(NCORES):
        b, g = c // GROUPS, c % GROUPS
        e0 = g * E
        in_maps.append({
            "qT": qTb[b],
            "kT": kTb[b],
            "vT": vTb[b],
            "wqT": np.ascontiguousarray(wq[e0:e0 + E, :].astype(NPBF16).T),
            "wkT": np.ascontiguousarray(wk[e0:e0 + E, :].astype(NPBF16).T),
            "wvT": np.ascontiguousarray(wv[e0:e0 + E, :].astype(NPBF16).T),
            "w0T": np.ascontiguousarray(w0[:, e0:e0 + E].astype(NPBF16).T),
        })
    return in_maps


def gather_out(results):
    out = np.zeros((B, S, D), dtype=np.float32)
    for c in range(NCORES):
        b = c // GROUPS
        out[b] += results[c]["out"].astype(np.float32)
    return out


def _install_ntff_hook_shim():
    """This image's antenv lacks axon_hooks; recreate it so trace=True works.

    Mirrors trn_agent_boot.trn_boot._ntff_profile_via_ctypes against
    /opt/axon/libaxon_pjrt.so.
    """
    import sys, types, ctypes, contextlib
    if "antenv.axon_hooks" in sys.modules:
        return
    mod = types.ModuleType("antenv.axon_hooks")
    mod._hook = None

    def set_axon_ntff_profile_hook(h):
        mod._hook = h

    def get_axon_ntff_profile_hook():
        return mod._hook

    mod.set_axon_ntff_profile_hook = set_axon_ntff_profile_hook
    mod.get_axon_ntff_profile_hook = get_axon_ntff_profile_hook
    sys.modules["antenv.axon_hooks"] = mod
    try:
        import antenv
        antenv.axon_hooks = mod
    except ImportError:
        pass

    so_path = "/opt/axon/libaxon_pjrt.so"
    try:
        lib = ctypes.CDLL(so_path)
        if not hasattr(lib, "axon_start_nrt_profile"):
            return
        lib.axon_start_nrt_profile.argtypes = [
            ctypes.POINTER(ctypes.c_int64), ctypes.c_size_t]
        lib.axon_start_nrt_profile.restype = ctypes.c_int64
        lib.axon_stop_nrt_profile.argtypes = [ctypes.c_char_p]
        lib.axon_stop_nrt_profile.restype = ctypes.c_int64
    except OSError:
        return

    @contextlib.contextmanager
    def _hook(output_dir, device_ids):
        import jax
        jax.devices()
        if device_ids:
            ids = (ctypes.c_int64 * len(device_ids))(*device_ids)
            rc = lib.axon_start_nrt_profile(ids, len(device_ids))
        else:
            rc = lib.axon_start_nrt_profile(None, 0)
        if rc != 0:
            raise RuntimeError(f"axon_start_nrt_profile rc={rc}")
        try:
            yield
        finally:
            n = lib.axon_stop_nrt_profile(str(output_dir).encode())
            print(f"profile: {n} file(s) written to {output_dir}")

    mod._hook = _hook


def kernel(q, k, v, wq, wk, wv, w0, _trace=False, _tmpdir=None):
    if _trace:
        _install_ntff_hook_shim()
    nc = _get_program()
    in_maps = make_in_maps(q, k, v, wq, wk, wv, w0)
    res = bass_utils.run_bass_kernel_spmd(
        nc, in_maps, core_ids=list(range(NCORES)),
        trace=_trace, tmpdir=_tmpdir)
    out = gather_out(res.results)
    if _trace:
        return out, res
    return out
